# revision 24
# baseline (speedup 1.0000x reference)
"""Trainium2 Bass kernel for the AttentionBlock problem.

Full inputs -> full output. Internally sharded across 8 NeuronCores:
core c computes output rows [1024*c, 1024*(c+1)) (sequence-parallel over
queries); every core receives the full x (2 MB) so no on-device
collectives are needed.

Per-core algorithm (N=8192 keys, Nq=1024 queries, d=64):
  Qs^T = (R/8)^T x_q^T, K^T = E^T x^T            (fp32 PE matmuls)
  per 512-query chunk qc:
    pass 1: m_q = max_k (Qs K^T)[q, k]            (bf16 PE + DVE reduce)
    pass 2 (augmented, transposed):
      lhsT = [K^T; 1] (65 x 128 blocks), rhs = [Qs^T; -m]  (65 x 512)
      S^T_shifted = matmul -> PSUM (fp32), exp on ACT -> P^T (fp16)
      out_aug^T += matmul(lhsT=x_aug_j [128 x 65] fp16, rhs=P^T_j)
    out = out_aug^T[0:64] / out_aug^T[64] (PE transpose + reciprocal + mul)

The ones column of x_aug makes row 64 of out_aug^T the softmax
denominator; the -m row of the augmented Qs^T applies the max shift
inside the matmul (softmax is shift-invariant, so bf16 max error only
moves the shift, never the result).
"""

import numpy as np
from contextlib import ExitStack

import concourse.bass as bass
import concourse.tile as tile
from concourse import bacc, mybir

N = 8192
D = 64
DA = D + 1
NCORES = 8
NQ = N // NCORES          # 1024 queries per core
NKB = N // 128            # 64 key blocks
NSC = N // 512            # 16 key chunks of 512
QC = 512                  # query chunk (pass-2 moving dim)
NQC = NQ // QC            # 2

ST_DT = "f16x3"           # score matmul dtype: f32 | f32r | f16x3
PV_DT = "f16"             # PV matmul dtype: f16 | f32r

F32 = mybir.dt.float32
F32R = mybir.dt.float32r
F16 = mybir.dt.float16
BF16 = mybir.dt.bfloat16


def build(st_dt=None, pv_dt=None):
    st_name = st_dt or ST_DT
    st_split = st_name == "f16x3"
    st_dt = {"f32": F32, "f32r": F32R, "f16x3": F16}[st_name]
    pv_dt = {"f16": F16, "f32r": F32R}[pv_dt or PV_DT]

    nc = bacc.Bacc("TRN2", target_bir_lowering=False, debug=False, num_devices=1)

    x_ap = nc.dram_tensor("x", [N, D], F32, kind="ExternalInput").ap()
    xT_ap = nc.dram_tensor("xT", [D, N], F32, kind="ExternalInput").ap()
    xqT_ap = nc.dram_tensor("xqT", [D, NQ], F32, kind="ExternalInput").ap()
    rp_ap = nc.dram_tensor("Rp", [D, D], F32, kind="ExternalInput").ap()
    e_ap = nc.dram_tensor("E", [D, D], F32, kind="ExternalInput").ap()
    id_ap = nc.dram_tensor("ident", [128, 128], F32, kind="ExternalInput").ap()
    ones_ap = nc.dram_tensor("ones", [1, N], F32, kind="ExternalInput").ap()
    ones16_ap = nc.dram_tensor("ones16", [1, N], F16, kind="ExternalInput").ap()
    DP = 72                   # x_aug block stride, 16-byte aligned in fp16
    xaug_ap = nc.dram_tensor("xaug", [N, DP], F16, kind="ExternalInput").ap()
    out_ap = nc.dram_tensor("out", [NQ, D], F32, kind="ExternalOutput").ap()

    with tile.TileContext(nc) as tc, ExitStack() as ctx:
        const = ctx.enter_context(tc.tile_pool(name="const", bufs=1))
        big = ctx.enter_context(tc.tile_pool(name="big", bufs=1))
        work = ctx.enter_context(tc.tile_pool(name="work", bufs=3))
        # PSUM budget (8 banks): ps1 [128,1024] x2 = 4, mm512 [128,512] x2 = 2,
        # po [65,512] x2 = 2.
        pp1 = ctx.enter_context(tc.tile_pool(name="pp1", bufs=2, space="PSUM"))
        pp = ctx.enter_context(tc.tile_pool(name="pp", bufs=2, space="PSUM"))
        pacc = ctx.enter_context(tc.tile_pool(name="pacc", bufs=2, space="PSUM"))

        # ---------------- input loads ----------------
        # small projection weights first so the first matmuls start early
        rp_sb = const.tile([D, D], F32)
        nc.sync.dma_start(rp_sb[:], rp_ap[:])
        e_sb = const.tile([D, D], F32)
        nc.sync.dma_start(e_sb[:], e_ap[:])
        xqt_sb = big.tile([D, NQ], F32)
        nc.sync.dma_start(xqt_sb[:], xqT_ap[:])
        xt_sb = big.tile([D, N], F32)
        for s in range(8):
            w = N // 8
            nc.sync.dma_start(xt_sb[:, s * w:(s + 1) * w], xT_ap[:, s * w:(s + 1) * w])
        ident = const.tile([128, 128], F32)
        nc.sync.dma_start(ident[:], id_ap[:])

        # x with ones column for the PV matmul, layout [128, (block, d_pad)]
        xaug_r = big.tile([128, NKB * DP], pv_dt)
        if pv_dt == F16:
            nc.sync.dma_start(
                xaug_r[:].rearrange("p (t d) -> p t d", d=DP),
                xaug_ap.rearrange("(t p) d -> p t d", p=128))
        else:
            xaug_f = big.tile([128, NKB * DA], F32)
            xaug_view_f = xaug_f[:].rearrange("p (t d) -> p t d", d=DA)
            nc.vector.memset(xaug_view_f[:, :, D:DA], 1.0)
            nc.sync.dma_start(xaug_view_f[:, :, 0:D],
                              x_ap.rearrange("(t p) d -> p t d", p=128))
            nc.vector.tensor_copy(xaug_r[:], xaug_f[:])
        xaug_v = xaug_r[:].rearrange("p (t d) -> p t d", d=DP)[:, :, 0:DA]

        # ---------------- projections ----------------
        # Qs^T first (2 chunks) so pass-1 lhsT is ready early.
        # K^T is split into 4 quarter tiles so pass-1 score matmuls (and
        # their DVE reductions) start after the first quarter instead of
        # waiting for the whole projection (Tile deps are tile-granular).
        NKQ = 4
        KW = N // NKQ
        qst_s = big.tile([DA, NQ], st_dt)
        qst_l = big.tile([D, NQ], F16, name="qst_l") if st_split else None
        qst_bf = qst_s if st_split else big.tile([D, NQ], BF16)
        for s in range(NQ // 512):
            pq_full = pp.tile([128, 512], F32, tag="mm512", name="pq")
            pq = pq_full[0:D, :]
            nc.tensor.matmul(pq[:], rp_sb[:], xqt_sb[:, s * 512:(s + 1) * 512],
                             start=True, stop=True)
            sl = slice(s * 512, (s + 1) * 512)
            if st_split:
                nc.scalar.copy(qst_s[0:D, sl], pq[:])
                nc.vector.tensor_tensor(
                    out=qst_l[:, sl], in0=pq[:], in1=qst_s[0:D, sl],
                    op=mybir.AluOpType.subtract)
            else:
                nc.vector.tensor_copy(qst_s[0:D, sl], pq[:])
                nc.scalar.copy(qst_bf[:, sl], pq[:])

        kt_ss = [big.tile([DA, KW], st_dt, name=f"kt_s{q}") for q in range(NKQ)]
        kt_ls = ([big.tile([D, KW], F16, name=f"kt_l{q}") for q in range(NKQ)]
                 if st_split else None)
        kt_bfs = (kt_ss if st_split
                  else [big.tile([D, KW], BF16, name=f"kt_b{q}") for q in range(NKQ)])
        for q in range(NKQ):
            qw = slice(q * KW, (q + 1) * KW)
            if st_dt == F32:
                nc.sync.dma_start(kt_ss[q][D:DA, :], ones_ap[:, qw])
            elif st_split:
                nc.sync.dma_start(kt_ss[q][D:DA, :].bitcast(F32),
                                  ones16_ap[:, qw].bitcast(F32))
            else:
                ones_f = const.tile([1, KW], F32, tag="ones_f", name="ones_f")
                nc.vector.memset(ones_f[:], 1.0)
                nc.vector.tensor_copy(kt_ss[q][D:DA, :], ones_f[:])
        for s in range(NSC):
            q, so = divmod(s, NSC // NKQ)
            pk_full = pp.tile([128, 512], F32, tag="mm512", name="pk")
            pk = pk_full[0:D, :]
            nc.tensor.matmul(pk[:], e_sb[:], xt_sb[:, s * 512:(s + 1) * 512],
                             start=True, stop=True)
            sl = slice(so * 512, (so + 1) * 512)
            if st_split:
                # hi part on ACT, residual on DVE; the hi part doubles as the
                # pass-1 score operand (fp16 hi is more accurate than bf16)
                nc.scalar.copy(kt_ss[q][0:D, sl], pk[:])
                nc.vector.tensor_tensor(
                    out=kt_ls[q][:, sl], in0=pk[:], in1=kt_ss[q][0:D, sl],
                    op=mybir.AluOpType.subtract)
            else:
                nc.vector.tensor_copy(kt_ss[q][0:D, sl], pk[:])
                nc.scalar.copy(kt_bfs[q][:, sl], pk[:])

        # -------- pass 1 for chunk 0, then pass 2 per chunk with the next
        # chunk's pass 1 interleaved into the j-loop. Engines execute a fixed
        # per-engine order, so emission order must keep chunk qc+1's max
        # reductions (DVE) flowing underneath chunk qc's pass 2 (PE/ACT)
        # without ever stalling the PE order on a ps1 slot.
        NRT = QC // 128                   # row-tiles per chunk (4)
        NG = NSC // 2                     # reduce groups per row-tile (8)
        mx_tiles = {}
        mxp_tiles = {}

        def emit_pass1_group(qc, gi):
            rt, g = divmod(gi, NG)
            if g == 0:
                mxp_tiles[qc] = work.tile([128, NG], F32, tag="mxp", name="mxp")
            mxp = mxp_tiles[qc]
            q0 = qc * QC + rt * 128
            ps1 = pp1.tile([128, 1024], F32, tag="ps1", name="ps1")
            for h in range(2):
                s = g * 2 + h
                kq, so = divmod(s, NSC // NKQ)
                nc.tensor.matmul(ps1[:, h * 512:(h + 1) * 512],
                                 qst_bf[0:D, q0:q0 + 128],
                                 kt_bfs[kq][0:D, so * 512:(so + 1) * 512],
                                 start=True, stop=True)
            nc.vector.reduce_max(mxp[:, g:g + 1], ps1[:],
                                 axis=mybir.AxisListType.X)
            if g == NG - 1:
                if qc not in mx_tiles:
                    mx_tiles[qc] = work.tile([128, NRT + 32], F32,
                                             tag="mx_all", name="mx_all")
                    nc.vector.memset(mx_tiles[qc][:], 0.0)
                nc.vector.reduce_max(mx_tiles[qc][:, rt:rt + 1], mxp[:],
                                     axis=mybir.AxisListType.X, negate=True)

        def emit_max_writeback(qc):
            # PSUM/SBUF reads must start at an aligned partition, so bring
            # each row-tile's -max to partition 0 with its own 32-wide
            # (non-degenerate) PE transpose of the zero-padded max tile,
            # then copy row 0 into qst_s row 64.
            for rt in range(NRT):
                pm_full = pp.tile([128, 512], F32, tag="mm512", name="pm")
                ps_m = pm_full[0:32, 0:128]
                nc.tensor.transpose(ps_m[:], mx_tiles[qc][:, rt:rt + 32],
                                    ident[:])
                sl = slice(qc * QC + rt * 128, qc * QC + (rt + 1) * 128)
                nc.vector.tensor_copy(qst_s[D:DA, sl], ps_m[0:1, :])

        for gi in range(NRT * NG):
            emit_pass1_group(0, gi)
        emit_max_writeback(0)

        for qc in range(NQC):
            # pass 2, software-pipelined at emission so the PE order is
            # S_0, S_1, PV_0, S_2, PV_1, ... (PE never waits on an exp)
            po = pacc.tile([DA, QC], F32, tag="po")

            def emit_st(j):
                ps = pp.tile([128, QC], F32, tag="mm512", name="ps_st")
                kq, jo = divmod(j, NKB // NKQ)
                blk = slice(jo * 128, (jo + 1) * 128)
                qsl = slice(qc * QC, (qc + 1) * QC)
                if st_split:
                    nc.tensor.matmul(ps[:], kt_ss[kq][:, blk], qst_s[:, qsl],
                                     start=True, stop=False)
                    nc.tensor.matmul(ps[:], kt_ls[kq][:, blk], qst_s[0:D, qsl],
                                     start=False, stop=False)
                    nc.tensor.matmul(ps[:], kt_ss[kq][0:D, blk], qst_l[:, qsl],
                                     start=False, stop=True)
                else:
                    nc.tensor.matmul(ps[:], kt_ss[kq][:, blk], qst_s[:, qsl],
                                     start=True, stop=True)
                return ps

            ps_cur = emit_st(0)
            for j in range(NKB):
                pt = work.tile([128, QC], pv_dt, tag="pt")
                nc.scalar.activation(pt[:], ps_cur[:],
                                     mybir.ActivationFunctionType.Exp)
                if j + 1 < NKB:
                    ps_cur = emit_st(j + 1)
                nc.tensor.matmul(po[:], xaug_v[:, j, :], pt[:],
                                 start=(j == 0), stop=(j == NKB - 1))
                iv = NKB // (NRT * NG)   # j-iters per pass-1 group
                if qc + 1 < NQC and j % iv == iv - 1:
                    emit_pass1_group(qc + 1, j // iv)
            if qc + 1 < NQC:
                emit_max_writeback(qc + 1)

            # normalize: out[q, :] = po[0:64, q] / po[64, q]
            ot = work.tile([DA, QC], F32, tag="ot")
            nc.vector.tensor_copy(ot[:], po[:])
            for h in range(QC // 128):
                ptr_full = pp.tile([128, 512], F32, tag="mm512", name="ptr")
                ps_t = ptr_full[:, 0:DA]
                nc.tensor.transpose(ps_t[:], ot[:, h * 128:(h + 1) * 128],
                                    ident[0:DA, 0:DA])
                recip = work.tile([128, 1], F32, tag="recip")
                nc.vector.reciprocal(recip[:], ps_t[:, D:DA])
                o_sb = work.tile([128, D], F32, tag="o_sb")
                nc.vector.tensor_scalar_mul(o_sb[:], ps_t[:, 0:D], recip[:])
                r0 = qc * QC + h * 128
                nc.sync.dma_start(out_ap[r0:r0 + 128, :], o_sb[:])

    nc.compile()
    return nc


_CACHE = {}


def _get_nc():
    key = (ST_DT, PV_DT)
    if key not in _CACHE:
        _CACHE[key] = build(*key)
    return _CACHE[key]


def kernel(x, rotation_params, entangle_params, _trace=False, _nc=None):
    from concourse.bass_utils import run_bass_kernel_spmd

    x = np.ascontiguousarray(x, dtype=np.float32)
    rp = np.ascontiguousarray(rotation_params, dtype=np.float32) / 8.0
    e = np.ascontiguousarray(entangle_params, dtype=np.float32)
    xT = np.ascontiguousarray(x.T)

    nc = _nc if _nc is not None else _get_nc()
    ones = np.ones((1, N), dtype=np.float32)
    xaug16 = np.zeros((N, 72), dtype=np.float16)
    xaug16[:, :D] = x.astype(np.float16)
    xaug16[:, D] = 1.0

    in_maps = []
    for c in range(NCORES):
        in_maps.append({
            "x": x,
            "xT": xT,
            "xqT": np.ascontiguousarray(xT[:, c * NQ:(c + 1) * NQ]),
            "Rp": rp,
            "E": e,
            "ident": np.eye(128, dtype=np.float32),
            "ones": ones,
            "ones16": ones.astype(np.float16),
            "xaug": xaug16,
        })
    res = run_bass_kernel_spmd(nc, in_maps, core_ids=list(range(NCORES)),
                               trace=_trace)
    out = np.concatenate([res.results[c]["out"] for c in range(NCORES)], axis=0)
    if _trace:
        return out, res
    return out


# revision 26
# speedup vs baseline: 1.3334x; 1.3334x over previous
"""Trainium2 Bass kernel for the AttentionBlock problem.

Full inputs -> full output. Internally sharded across 8 NeuronCores:
core c computes output rows [1024*c, 1024*(c+1)) (sequence-parallel over
queries); every core receives the full x (2 MB) so no on-device
collectives are needed.

Per-core algorithm (N=8192 keys, Nq=1024 queries, d=64):
  Qs^T = (R/8)^T x_q^T, K^T = E^T x^T            (fp32 PE matmuls)
  per 512-query chunk qc:
    pass 1: m_q = max_k (Qs K^T)[q, k]            (bf16 PE + DVE reduce)
    pass 2 (augmented, transposed):
      lhsT = [K^T; 1] (65 x 128 blocks), rhs = [Qs^T; -m]  (65 x 512)
      S^T_shifted = matmul -> PSUM (fp32), exp on ACT -> P^T (fp16)
      out_aug^T += matmul(lhsT=x_aug_j [128 x 65] fp16, rhs=P^T_j)
    out = out_aug^T[0:64] / out_aug^T[64] (PE transpose + reciprocal + mul)

The ones column of x_aug makes row 64 of out_aug^T the softmax
denominator; the -m row of the augmented Qs^T applies the max shift
inside the matmul (softmax is shift-invariant, so bf16 max error only
moves the shift, never the result).
"""

import numpy as np
from contextlib import ExitStack

import concourse.bass as bass
import concourse.tile as tile
from concourse import bacc, mybir

N = 8192
D = 64
DA = D + 1
NCORES = 8
NQ = N // NCORES          # 1024 queries per core
NKB = N // 128            # 64 key blocks
NSC = N // 512            # 16 key chunks of 512
QC = 512                  # query chunk (pass-2 moving dim)
NQC = NQ // QC            # 2

ST_DT = "f16x2p"          # score matmul: f32 | f32r | f16x3 | f16x2p (packed)
PV_DT = "f16"             # PV matmul dtype: f16 | f32r

F32 = mybir.dt.float32
F32R = mybir.dt.float32r
F16 = mybir.dt.float16
BF16 = mybir.dt.bfloat16


def build(st_dt=None, pv_dt=None):
    st_name = st_dt or ST_DT
    st_split = st_name in ("f16x3", "f16x2p")
    st_pack = st_name == "f16x2p"
    st_dt = {"f32": F32, "f32r": F32R, "f16x3": F16, "f16x2p": F16}[st_name]
    pv_dt = {"f16": F16, "f32r": F32R}[pv_dt or PV_DT]

    nc = bacc.Bacc("TRN2", target_bir_lowering=False, debug=False, num_devices=1)

    x_ap = nc.dram_tensor("x", [N, D], F32, kind="ExternalInput").ap()
    xT_ap = nc.dram_tensor("xT", [D, N], F32, kind="ExternalInput").ap()
    xqT_ap = nc.dram_tensor("xqT", [D, NQ], F32, kind="ExternalInput").ap()
    rp_ap = nc.dram_tensor("Rp", [D, D], F32, kind="ExternalInput").ap()
    e_ap = nc.dram_tensor("E", [D, D], F32, kind="ExternalInput").ap()
    id_ap = nc.dram_tensor("ident", [128, 128], F32, kind="ExternalInput").ap()
    ones_ap = nc.dram_tensor("ones", [1, N], F32, kind="ExternalInput").ap()
    ones16_ap = nc.dram_tensor("ones16", [1, N], F16, kind="ExternalInput").ap()
    DP = 72                   # x_aug block stride, 16-byte aligned in fp16
    xaug_ap = nc.dram_tensor("xaug", [N, DP], F16, kind="ExternalInput").ap()
    out_ap = nc.dram_tensor("out", [NQ, D], F32, kind="ExternalOutput").ap()

    with tile.TileContext(nc) as tc, ExitStack() as ctx:
        const = ctx.enter_context(tc.tile_pool(name="const", bufs=1))
        big = ctx.enter_context(tc.tile_pool(name="big", bufs=1))
        work = ctx.enter_context(tc.tile_pool(name="work", bufs=3))
        # PSUM budget (8 banks): ps1 [128,1024] x2 = 4, mm512 [128,512] x2 = 2,
        # po [65,512] x2 = 2.
        pp1 = ctx.enter_context(tc.tile_pool(name="pp1", bufs=2, space="PSUM"))
        pp = ctx.enter_context(tc.tile_pool(name="pp", bufs=2, space="PSUM"))
        pacc = ctx.enter_context(tc.tile_pool(name="pacc", bufs=2, space="PSUM"))

        # ---------------- input loads ----------------
        # small projection weights first so the first matmuls start early
        rp_sb = const.tile([D, D], F32)
        nc.sync.dma_start(rp_sb[:], rp_ap[:])
        e_sb = const.tile([D, D], F32)
        nc.sync.dma_start(e_sb[:], e_ap[:])
        xqt_sb = big.tile([D, NQ], F32)
        nc.sync.dma_start(xqt_sb[:], xqT_ap[:])
        xt_sb = big.tile([D, N], F32)
        for s in range(8):
            w = N // 8
            nc.sync.dma_start(xt_sb[:, s * w:(s + 1) * w], xT_ap[:, s * w:(s + 1) * w])
        ident = const.tile([128, 128], F32)
        nc.sync.dma_start(ident[:], id_ap[:])

        # x with ones column for the PV matmul, layout [128, (block, d_pad)]
        xaug_r = big.tile([128, NKB * DP], pv_dt)
        if pv_dt == F16:
            nc.sync.dma_start(
                xaug_r[:].rearrange("p (t d) -> p t d", d=DP),
                xaug_ap.rearrange("(t p) d -> p t d", p=128))
        else:
            xaug_f = big.tile([128, NKB * DA], F32)
            xaug_view_f = xaug_f[:].rearrange("p (t d) -> p t d", d=DA)
            nc.vector.memset(xaug_view_f[:, :, D:DA], 1.0)
            nc.sync.dma_start(xaug_view_f[:, :, 0:D],
                              x_ap.rearrange("(t p) d -> p t d", p=128))
            nc.vector.tensor_copy(xaug_r[:], xaug_f[:])
        xaug_v = xaug_r[:].rearrange("p (t d) -> p t d", d=DP)[:, :, 0:DA]

        # ---------------- projections ----------------
        # Qs^T first (2 chunks) so pass-1 lhsT is ready early.
        # K^T is split into 4 quarter tiles so pass-1 score matmuls (and
        # their DVE reductions) start after the first quarter instead of
        # waiting for the whole projection (Tile deps are tile-granular).
        NKQ = 4
        KW = N // NKQ
        qst_s = big.tile([DA, NQ], st_dt)
        qst_l = (big.tile([DA if st_pack else D, NQ], F16, name="qst_l")
                 if st_split else None)
        qst_hh = big.tile([128, NQ], F16, name="qst_hh") if st_pack else None
        qst_bf = qst_s if st_split else big.tile([D, NQ], BF16)
        for s in range(NQ // 512):
            pq_full = pp.tile([128, 512], F32, tag="mm512", name="pq")
            pq = pq_full[0:D, :]
            nc.tensor.matmul(pq[:], rp_sb[:], xqt_sb[:, s * 512:(s + 1) * 512],
                             start=True, stop=True)
            sl = slice(s * 512, (s + 1) * 512)
            if st_split:
                nc.scalar.copy(qst_s[0:D, sl], pq[:])
                nc.vector.tensor_tensor(
                    out=qst_l[0:D, sl], in0=pq[:], in1=qst_s[0:D, sl],
                    op=mybir.AluOpType.subtract)
                if st_pack:
                    nc.scalar.copy(qst_hh[0:D, sl], pq[:])
                    nc.scalar.copy(qst_hh[D:2 * D, sl], pq[:])
            else:
                nc.vector.tensor_copy(qst_s[0:D, sl], pq[:])
                nc.scalar.copy(qst_bf[:, sl], pq[:])

        kt_ss = [big.tile([DA, KW], st_dt, name=f"kt_s{q}") for q in range(NKQ)]
        kt_ls = ([big.tile([D, KW], F16, name=f"kt_l{q}") for q in range(NKQ)]
                 if st_split and not st_pack else None)
        kt_hl = ([big.tile([128, KW], F16, name=f"kt_hl{q}") for q in range(NKQ)]
                 if st_pack else None)
        kt_bfs = (kt_ss if st_split
                  else [big.tile([D, KW], BF16, name=f"kt_b{q}") for q in range(NKQ)])
        for q in range(NKQ):
            qw = slice(q * KW, (q + 1) * KW)
            if st_dt == F32:
                nc.sync.dma_start(kt_ss[q][D:DA, :], ones_ap[:, qw])
            elif st_split:
                nc.sync.dma_start(kt_ss[q][D:DA, :].bitcast(F32),
                                  ones16_ap[:, qw].bitcast(F32))
            else:
                ones_f = const.tile([1, KW], F32, tag="ones_f", name="ones_f")
                nc.vector.memset(ones_f[:], 1.0)
                nc.vector.tensor_copy(kt_ss[q][D:DA, :], ones_f[:])
        for s in range(NSC):
            q, so = divmod(s, NSC // NKQ)
            pk_full = pp.tile([128, 512], F32, tag="mm512", name="pk")
            pk = pk_full[0:D, :]
            nc.tensor.matmul(pk[:], e_sb[:], xt_sb[:, s * 512:(s + 1) * 512],
                             start=True, stop=True)
            sl = slice(so * 512, (so + 1) * 512)
            if st_split:
                # hi part on ACT, residual on DVE; the hi part doubles as the
                # pass-1 score operand (fp16 hi is more accurate than bf16)
                nc.scalar.copy(kt_ss[q][0:D, sl], pk[:])
                if st_pack:
                    nc.scalar.copy(kt_hl[q][0:D, sl], pk[:])
                    nc.vector.tensor_tensor(
                        out=kt_hl[q][D:2 * D, sl], in0=pk[:],
                        in1=kt_ss[q][0:D, sl], op=mybir.AluOpType.subtract)
                else:
                    nc.vector.tensor_tensor(
                        out=kt_ls[q][:, sl], in0=pk[:], in1=kt_ss[q][0:D, sl],
                        op=mybir.AluOpType.subtract)
            else:
                nc.vector.tensor_copy(kt_ss[q][0:D, sl], pk[:])
                nc.scalar.copy(kt_bfs[q][:, sl], pk[:])

        # -------- pass 1 for chunk 0, then pass 2 per chunk with the next
        # chunk's pass 1 interleaved into the j-loop. Engines execute a fixed
        # per-engine order, so emission order must keep chunk qc+1's max
        # reductions (DVE) flowing underneath chunk qc's pass 2 (PE/ACT)
        # without ever stalling the PE order on a ps1 slot.
        NRT = QC // 128                   # row-tiles per chunk (4)
        NG = NSC // 2                     # reduce groups per row-tile (8)
        mx_tiles = {}
        mxp_tiles = {}

        def emit_pass1_group(qc, gi):
            rt, g = divmod(gi, NG)
            if g == 0:
                mxp_tiles[qc] = work.tile([128, NG], F32, tag="mxp", name="mxp")
            mxp = mxp_tiles[qc]
            q0 = qc * QC + rt * 128
            ps1 = pp1.tile([128, 1024], F32, tag="ps1", name="ps1")
            for h in range(2):
                s = g * 2 + h
                kq, so = divmod(s, NSC // NKQ)
                nc.tensor.matmul(ps1[:, h * 512:(h + 1) * 512],
                                 qst_bf[0:D, q0:q0 + 128],
                                 kt_bfs[kq][0:D, so * 512:(so + 1) * 512],
                                 start=True, stop=True)
            nc.vector.reduce_max(mxp[:, g:g + 1], ps1[:],
                                 axis=mybir.AxisListType.X)
            if g == NG - 1:
                if qc not in mx_tiles:
                    mx_tiles[qc] = work.tile([128, NRT + 32], F32,
                                             tag="mx_all", name="mx_all")
                    nc.vector.memset(mx_tiles[qc][:], 0.0)
                nc.vector.reduce_max(mx_tiles[qc][:, rt:rt + 1], mxp[:],
                                     axis=mybir.AxisListType.X, negate=True)

        def emit_max_writeback(qc):
            # PSUM/SBUF reads must start at an aligned partition, so bring
            # each row-tile's -max to partition 0 with its own 32-wide
            # (non-degenerate) PE transpose of the zero-padded max tile,
            # then copy row 0 into qst_s row 64.
            for rt in range(NRT):
                pm_full = pp.tile([128, 512], F32, tag="mm512", name="pm")
                ps_m = pm_full[0:32, 0:128]
                nc.tensor.transpose(ps_m[:], mx_tiles[qc][:, rt:rt + 32],
                                    ident[:])
                sl = slice(qc * QC + rt * 128, qc * QC + (rt + 1) * 128)
                nc.vector.tensor_copy(
                    (qst_l if st_pack else qst_s)[D:DA, sl], ps_m[0:1, :])

        for gi in range(NRT * NG):
            emit_pass1_group(0, gi)
        emit_max_writeback(0)

        for qc in range(NQC):
            # pass 2, software-pipelined at emission so the PE order is
            # S_0, S_1, PV_0, S_2, PV_1, ... (PE never waits on an exp)
            po = pacc.tile([DA, QC], F32, tag="po")

            def emit_st(j):
                ps = pp.tile([128, QC], F32, tag="mm512", name="ps_st")
                kq, jo = divmod(j, NKB // NKQ)
                blk = slice(jo * 128, (jo + 1) * 128)
                qsl = slice(qc * QC, (qc + 1) * QC)
                if st_pack:
                    nc.tensor.matmul(ps[:], kt_hl[kq][:, blk], qst_hh[:, qsl],
                                     start=True, stop=False)
                    nc.tensor.matmul(ps[:], kt_ss[kq][:, blk], qst_l[:, qsl],
                                     start=False, stop=True)
                elif st_split:
                    nc.tensor.matmul(ps[:], kt_ss[kq][:, blk], qst_s[:, qsl],
                                     start=True, stop=False)
                    nc.tensor.matmul(ps[:], kt_ls[kq][:, blk], qst_s[0:D, qsl],
                                     start=False, stop=False)
                    nc.tensor.matmul(ps[:], kt_ss[kq][0:D, blk], qst_l[:, qsl],
                                     start=False, stop=True)
                else:
                    nc.tensor.matmul(ps[:], kt_ss[kq][:, blk], qst_s[:, qsl],
                                     start=True, stop=True)
                return ps

            ps_cur = emit_st(0)
            for j in range(NKB):
                pt = work.tile([128, QC], pv_dt, tag="pt")
                nc.scalar.activation(pt[:], ps_cur[:],
                                     mybir.ActivationFunctionType.Exp)
                if j + 1 < NKB:
                    ps_cur = emit_st(j + 1)
                nc.tensor.matmul(po[:], xaug_v[:, j, :], pt[:],
                                 start=(j == 0), stop=(j == NKB - 1))
                iv = NKB // (NRT * NG)   # j-iters per pass-1 group
                if qc + 1 < NQC and j % iv == iv - 1:
                    emit_pass1_group(qc + 1, j // iv)
            if qc + 1 < NQC:
                emit_max_writeback(qc + 1)

            # normalize: out[q, :] = po[0:64, q] / po[64, q]
            ot = work.tile([DA, QC], F32, tag="ot")
            nc.vector.tensor_copy(ot[:], po[:])
            for h in range(QC // 128):
                ptr_full = pp.tile([128, 512], F32, tag="mm512", name="ptr")
                ps_t = ptr_full[:, 0:DA]
                nc.tensor.transpose(ps_t[:], ot[:, h * 128:(h + 1) * 128],
                                    ident[0:DA, 0:DA])
                recip = work.tile([128, 1], F32, tag="recip")
                nc.vector.reciprocal(recip[:], ps_t[:, D:DA])
                o_sb = work.tile([128, D], F32, tag="o_sb")
                nc.vector.tensor_scalar_mul(o_sb[:], ps_t[:, 0:D], recip[:])
                r0 = qc * QC + h * 128
                nc.sync.dma_start(out_ap[r0:r0 + 128, :], o_sb[:])

    nc.compile()
    return nc


_CACHE = {}


def _get_nc():
    key = (ST_DT, PV_DT)
    if key not in _CACHE:
        _CACHE[key] = build(*key)
    return _CACHE[key]


def kernel(x, rotation_params, entangle_params, _trace=False, _nc=None):
    from concourse.bass_utils import run_bass_kernel_spmd

    x = np.ascontiguousarray(x, dtype=np.float32)
    rp = np.ascontiguousarray(rotation_params, dtype=np.float32) / 8.0
    e = np.ascontiguousarray(entangle_params, dtype=np.float32)
    xT = np.ascontiguousarray(x.T)

    nc = _nc if _nc is not None else _get_nc()
    ones = np.ones((1, N), dtype=np.float32)
    xaug16 = np.zeros((N, 72), dtype=np.float16)
    xaug16[:, :D] = x.astype(np.float16)
    xaug16[:, D] = 1.0

    in_maps = []
    for c in range(NCORES):
        in_maps.append({
            "x": x,
            "xT": xT,
            "xqT": np.ascontiguousarray(xT[:, c * NQ:(c + 1) * NQ]),
            "Rp": rp,
            "E": e,
            "ident": np.eye(128, dtype=np.float32),
            "ones": ones,
            "ones16": ones.astype(np.float16),
            "xaug": xaug16,
        })
    res = run_bass_kernel_spmd(nc, in_maps, core_ids=list(range(NCORES)),
                               trace=_trace)
    out = np.concatenate([res.results[c]["out"] for c in range(NCORES)], axis=0)
    if _trace:
        return out, res
    return out
